# revision 14
# baseline (speedup 1.0000x reference)
"""4-layer LSTM (B=64, T=1024, F=256, H=512) on 8 Trainium2 NeuronCores.

Strategy: layer-pipeline across cores. Core l (l=0..3) runs LSTM layer l for
the full batch; cores 4-7 run the same SPMD program with zero weights (their
hidden state is provably exactly zero). The sequence is processed in chunks
of C timesteps; each core ships its per-step transposed hidden states
(h.T, bf16) to the next core via one 4-rank AllGather per chunk tick
(groups [0-3] and [4-7]), with a 2-tick pipeline skew so the collective is
fully overlapped with compute.

Inside a step, gate pre-activations for all 4 gate-chunks are computed as
col-group-packed matmuls (M=64 batch in each 64-wide column group, two
concurrent groups). All matmul moving operands are bf16 (fp32 moving costs
4 cycles/row on the PE). The combined bias lands in PSUM via a K=2 one-hot
matmul, so the activations (sigmoid/tanh) read PSUM directly with no
elementwise bias add on the critical path. Per-step PE order is:
bias MMs + input-projection MMs first (no dependency on h, so they overlap
the previous step's elementwise chain), then the recurrent h@W_hh MMs,
grouped per PSUM bank so bank 0's nonlinearity+cell update overlaps bank 1's
matmul stream.

Core 0's AllGather slot carries garbage (core 3's h); it is neutralized by
placing core 0's real W_ih rows only at the x k-slots (256:512) and zeros at
the h k-slots (0:256), with x added into in_t columns 64:128.
"""

import sys

sys.path.insert(0, "/opt/trn_rl_repo")

import numpy as np
import ml_dtypes

import concourse.bass as bass
import concourse.mybir as mybir
from concourse import bacc
from concourse.tile import TileContext
from concourse.bass import ds
from concourse.masks import make_identity

BF16 = ml_dtypes.bfloat16

B, F, H, L = 64, 256, 512, 4
G = 4 * H  # 2048
NCORES = 8
SKEW = 2  # ticks between producer scan and consumer scan
T_FULL = 1024
C_DEFAULT = 16

_BUILD_CACHE = {}


def _gate_perm():
    """Permutation of the 4H gate columns.

    Chunk q (q=0..3, 512 cols) = [i, f, o, g] for h-dims 128q..128q+127.
    Original gate order in the reference is i, f, g, o (each H wide).
    """
    perm = np.empty(G, dtype=np.int64)
    for q in range(4):
        base = 512 * q
        hd = 128 * q + np.arange(128)
        perm[base + 0:base + 128] = 0 * H + hd  # i
        perm[base + 128:base + 256] = 1 * H + hd  # f
        perm[base + 256:base + 384] = 3 * H + hd  # o
        perm[base + 384:base + 512] = 2 * H + hd  # g
    return perm


def build(T, C, mode="real", reps=1):
    """Build and finalize the 8-core SPMD Bass program.

    mode="real": normal kernel.
    mode="sim": collectives replaced by local DMA (for TimelineSim).
    mode="timing": xT/out are internal DRAM (tiny host I/O), loop repeated
    `reps` times for wall-clock amplification; real collectives.
    """
    NC_CH = T // C
    NTICKS = NC_CH + SKEW * (L - 1)
    f32 = mybir.dt.float32
    bf16 = mybir.dt.bfloat16

    nc = bacc.Bacc("TRN2", target_bir_lowering=False, debug=False,
                   num_devices=NCORES)

    w_in_T = nc.declare_dram_parameter("w_in_T", [128, 4, G], bf16, isOutput=False)
    w_hh_T = nc.declare_dram_parameter("w_hh_T", [128, 4, G], bf16, isOutput=False)
    bias_d = nc.declare_dram_parameter("bias", [2, 2, 512], bf16, isOutput=False)
    kill_d = nc.declare_dram_parameter("kill", [128, NTICKS], f32, isOutput=False)
    hmask_d = nc.declare_dram_parameter("hmask", [128, 1], f32, isOutput=False)
    ones01_d = nc.declare_dram_parameter("ones01", [2, 128], bf16, isOutput=False)
    if mode == "timing":
        xT_d = nc.dram_tensor("xT_int", [128, T, 2, 64], bf16)
        out_d = nc.dram_tensor("out_int", [B, T, H], f32)
        nc.declare_dram_parameter("xT", [128, 2, 2, 64], bf16, isOutput=False)
        small_out = nc.declare_dram_parameter("out", [B, 2, H], f32, isOutput=True)
    else:
        xT_d = nc.declare_dram_parameter("xT", [128, T, 2, 64], bf16, isOutput=False)
        out_d = nc.declare_dram_parameter("out", [B, T, H], f32, isOutput=True)

    # chunk-sized h.T payload: [128 part, C steps, 2 banks, 128]
    ship_shape = [128, C, 2, 128]
    send_bufs = [nc.dram_tensor(f"send{p}", ship_shape, bf16) for p in range(2)]
    # Local (non-shared) output: shared-output collectives need >4-core
    # groups; with 4-core groups each rank gets its own gathered copy.
    ag_outs = [
        nc.dram_tensor(f"agout{p}", [4 * 128, C, 2, 128], bf16)
        for p in range(2)
    ]

    with TileContext(nc) as tc:
        with (
            tc.tile_pool(name="const", bufs=1) as constp,
            tc.tile_pool(name="state", bufs=1) as statep,
            tc.tile_pool(name="inp", bufs=2) as inp,
            tc.tile_pool(name="stage", bufs=2) as stagep,
            tc.tile_pool(name="ew", bufs=3) as ewp,
            tc.tile_pool(name="gpsum", bufs=2, space="PSUM") as gpsum,
            tc.tile_pool(name="trpsum", bufs=2, space="PSUM") as trpsum,
        ):
            # ---- constants ----
            w_in_sb = constp.tile([128, 4, G], bf16)
            nc.sync.dma_start(out=w_in_sb[:], in_=w_in_T[:, :, :])
            w_hh_sb = constp.tile([128, 4, G], bf16)
            nc.sync.dma_start(out=w_hh_sb[:], in_=w_hh_T[:, :, :])
            bias_sb = constp.tile([2, 2, 512], bf16)
            nc.sync.dma_start(out=bias_sb[:], in_=bias_d[:, :, :])
            kill_sb = constp.tile([128, NTICKS], f32)
            nc.sync.dma_start(out=kill_sb[:], in_=kill_d[:, :])
            hmask_sb = constp.tile([128, 1], f32)
            nc.sync.dma_start(out=hmask_sb[:], in_=hmask_d[:, :])
            ident = constp.tile([128, 128], f32)
            make_identity(nc, ident[:])
            # one-hot stationary for the bias matmul: row 0 selects psum
            # partitions 0-63 (gate chunk b), row 1 partitions 64-127
            # (gate chunk b+2)
            ones01 = constp.tile([2, 128], bf16)
            nc.sync.dma_start(out=ones01[:], in_=ones01_d[:, :])

            # zero both send buffers (AG ticks 0 and 1 read pre-scan content)
            zt = constp.tile([128, C * 256], bf16)
            nc.vector.memset(zt[:], 0.0)
            for p in range(2):
                nc.sync.dma_start(out=send_bufs[p][:, :, :, :],
                                  in_=zt[:].rearrange("p (c b k) -> p c b k",
                                                      c=C, b=2))

            # ---- persistent state ----
            c_state = statep.tile([128, 2, 128], f32)
            hT_state = statep.tile([128, 2, 128], bf16)
            nc.vector.memset(c_state[:], 0.0)
            nc.vector.memset(hT_state[:], 0.0)

            prev = (nc.gpsimd.partition_id() + 3) % 4

            for tick in range(reps * NTICKS):
                tick = tick % NTICKS
                par = tick % 2
                # ---- collective: everyone ships its previous chunk ----
                if mode == "sim":
                    nc.gpsimd.dma_start(out=ag_outs[par][0:128, :, :, :],
                                        in_=send_bufs[par][:, :, :, :])
                else:
                    nc.gpsimd.collective_compute(
                        "AllGather", mybir.AluOpType.bypass,
                        replica_groups=[[0, 1, 2, 3], [4, 5, 6, 7]],
                        ins=[send_bufs[par].ap().opt()],
                        outs=[ag_outs[par].ap().opt()],
                    )
                # ---- IN fill: slot (rank-1 mod 4) of this AG + own xT chunk
                in_t = inp.tile([128, C, 2, 128], bf16, tag="IN")
                nc.gpsimd.dma_start(out=in_t[:],
                                    in_=ag_outs[par][ds(prev * 128, 128), :, :, :])
                # core 0's slot carries core 3's h — mask it to zero there
                nc.vector.tensor_scalar_mul(in_t[:], in_t[:],
                                            hmask_sb[:, 0:1])
                if tick < NC_CH:
                    x_t = inp.tile([128, C, 2, 64], bf16, tag="INX")
                    nc.sync.dma_start(out=x_t[:],
                                      in_=xT_d[:, tick * C:(tick + 1) * C, :, :])
                    nc.vector.tensor_add(in_t[:, :, :, 64:128],
                                         in_t[:, :, :, 64:128], x_t[:])
                in_cur = in_t

                # ---- kill garbage state at tick boundaries ----
                nc.vector.tensor_scalar_mul(c_state[:], c_state[:],
                                            kill_sb[:, tick:tick + 1])
                nc.vector.tensor_scalar_mul(hT_state[:], hT_state[:],
                                            kill_sb[:, tick:tick + 1])

                ship_t = stagep.tile([128, C, 2, 128], bf16, tag="SHIP")
                out_t = stagep.tile([128, C, 2, 128], f32, tag="OUT")

                def flush_pending(pt):
                    """Transpose step pt's h (both banks) into hT_state +
                    ship. Emitted *after* the next step's bias/x-proj MMs so
                    the PE stall on the elementwise chain is filled."""
                    for b in range(2):
                        tr = trpsum.tile([128, 128], f32, tag=f"tr{b}",
                                         name=f"tr{b}_{tick}_{pt}")
                        nc.tensor.transpose(tr[:], out_t[:, pt, b, :], ident[:])
                        nc.vector.tensor_copy(hT_state[:, b, :], tr[:])
                        nc.scalar.activation(ship_t[:, pt, b, :], tr[:],
                                             mybir.ActivationFunctionType.Copy)

                pending = None
                for t in range(C):
                    g_ps = [gpsum.tile([128, 512], f32, tag=f"g{q}",
                                       name=f"g{q}_{tick}_{t}")
                            for q in range(2)]
                    # ---- bias + input-projection matmuls (no h dependency;
                    # these overlap the previous step's elementwise chain) ----
                    for b in range(2):
                        nc.tensor.matmul(g_ps[b][0:128, :], ones01[:],
                                         bias_sb[:, b, :],
                                         start=True, stop=False)
                    for b in range(2):
                        for j in range(4):
                            lhsT = in_cur[:, t, j % 2,
                                          64 * (j // 2):64 * (j // 2) + 64]
                            for half in range(2):
                                q = b + 2 * half
                                nc.tensor.matmul(
                                    g_ps[b][64 * half:64 * half + 64, :],
                                    lhsT,
                                    w_in_sb[:, j, 512 * q:512 * q + 512],
                                    start=False, stop=False,
                                    tile_position=(0, 64 * half),
                                )
                    # ---- previous step's transposes (fills the PE queue
                    # while its elementwise chain finishes) ----
                    if pending is not None:
                        flush_pending(pending)
                    # ---- recurrent matmuls, bank 0 complete first ----
                    for b in range(2):
                        for idx, jj in enumerate((0, 2, 1, 3)):
                            lhsT = hT_state[:, jj % 2,
                                            64 * (jj // 2):64 * (jj // 2) + 64]
                            last = (idx == 3)
                            for half in range(2):
                                q = b + 2 * half
                                nc.tensor.matmul(
                                    g_ps[b][64 * half:64 * half + 64, :],
                                    lhsT,
                                    w_hh_sb[:, jj, 512 * q:512 * q + 512],
                                    start=False, stop=last,
                                    tile_position=(0, 64 * half),
                                )
                    # ---- elementwise per bank, straight from PSUM ----
                    for b in range(2):
                        sig = ewp.tile([128, 384], f32, tag=f"sig{b}")
                        nc.scalar.activation(sig[:], g_ps[b][:, 0:384],
                                             mybir.ActivationFunctionType.Sigmoid)
                        tg = ewp.tile([128, 128], f32, tag=f"tg{b}")
                        nc.scalar.activation(tg[:], g_ps[b][:, 384:512],
                                             mybir.ActivationFunctionType.Tanh)
                        t1 = ewp.tile([128, 128], f32, tag=f"t1{b}")
                        nc.vector.tensor_mul(t1[:], sig[:, 128:256],
                                             c_state[:, b, :])
                        t2 = ewp.tile([128, 128], f32, tag=f"t2{b}")
                        nc.vector.tensor_mul(t2[:], sig[:, 0:128], tg[:])
                        nc.vector.tensor_add(c_state[:, b, :], t1[:], t2[:])
                        tc_t = ewp.tile([128, 128], f32, tag=f"tc{b}")
                        nc.scalar.activation(tc_t[:], c_state[:, b, :],
                                             mybir.ActivationFunctionType.Tanh)
                        nc.vector.tensor_mul(out_t[:, t, b, :],
                                             sig[:, 256:384], tc_t[:])
                    pending = t
                flush_pending(pending)

                # ---- ship chunk for AG at tick+2 ----
                nc.sync.dma_start(out=send_bufs[par][:, :, :, :], in_=ship_t[:])
                # ---- write output chunk (core-3 semantics) ----
                ot = tick - SKEW * (L - 1)
                if 0 <= ot < NC_CH:
                    nc.sync.dma_start(
                        out=out_d[0:64, ot * C:(ot + 1) * C, 0:256],
                        in_=out_t[0:64, :, :, :])
                    nc.sync.dma_start(
                        out=out_d[0:64, ot * C:(ot + 1) * C, 256:512],
                        in_=out_t[64:128, :, :, :])
            if mode == "timing":
                zo = constp.tile([64, 2, 512], f32, name="zo")
                nc.vector.memset(zo[:], 0.0)
                nc.sync.dma_start(out=small_out[:, :, :], in_=zo[:])

    nc.finalize()
    return nc


def build_null(T, C):
    """Null program with identical external I/O — for timing calibration."""
    NC_CH = T // C
    NTICKS = NC_CH + SKEW * (L - 1)
    f32 = mybir.dt.float32
    bf16 = mybir.dt.bfloat16
    nc = bacc.Bacc("TRN2", target_bir_lowering=False, debug=False,
                   num_devices=NCORES)
    nc.declare_dram_parameter("w_in_T", [128, 4, G], bf16, isOutput=False)
    nc.declare_dram_parameter("w_hh_T", [128, 4, G], bf16, isOutput=False)
    nc.declare_dram_parameter("bias", [2, 2, 512], bf16, isOutput=False)
    nc.declare_dram_parameter("xT", [128, T, 2, 64], bf16, isOutput=False)
    kill_d = nc.declare_dram_parameter("kill", [128, NTICKS], f32,
                                       isOutput=False)
    nc.declare_dram_parameter("hmask", [128, 1], f32, isOutput=False)
    nc.declare_dram_parameter("ones01", [2, 128], bf16, isOutput=False)
    out_d = nc.declare_dram_parameter("out", [B, T, H], f32, isOutput=True)
    with TileContext(nc) as tc:
        with tc.tile_pool(name="p", bufs=2) as pool:
            t = pool.tile([128, NTICKS], f32)
            nc.sync.dma_start(out=t[:, 0:NTICKS], in_=kill_d[:, :])
            nc.sync.dma_start(out=out_d[0:64, 0:1, 0:NTICKS],
                              in_=t[0:64, 0:NTICKS].rearrange(
                                  "p (a b) -> p a b", a=1))
    nc.finalize()
    return nc


def _prep_core_inputs(x_sh, weights, T, C):
    """Build the 8 per-core input maps from full (already shifted) inputs."""
    NC_CH = T // C
    NTICKS = NC_CH + SKEW * (L - 1)
    perm = _gate_perm()

    zeros_xT = np.zeros([128, T, 2, 64], dtype=BF16)
    in_maps = []
    for core in range(NCORES):
        if core < L:
            W_ih, W_hh, b_ih, b_hh = weights[core]
            WiT = W_ih.T[:, perm].astype(np.float32)  # [F_in, G] permuted
            if WiT.shape[0] < 512:
                # core 0: real rows at the x k-slots (256:512); the h k-slots
                # (0:256) are zero so the garbage AllGather slot is harmless
                WiT = np.concatenate(
                    [np.zeros((512 - WiT.shape[0], G), np.float32), WiT], axis=0)
            WhT = W_hh.T[:, perm].astype(np.float32)
            bvec = (b_ih + b_hh)[perm].astype(np.float32)
        else:
            WiT = np.zeros((512, G), np.float32)
            WhT = np.zeros((512, G), np.float32)
            bvec = np.zeros(G, np.float32)

        w_in_T = WiT.reshape(4, 128, G).transpose(1, 0, 2).astype(BF16)
        w_hh_T = WhT.reshape(4, 128, G).transpose(1, 0, 2).astype(BF16)
        bias = np.zeros((2, 2, 512), np.float32)
        for b in range(2):
            bias[0, b, :] = bvec[512 * b:512 * b + 512]
            bias[1, b, :] = bvec[512 * (b + 2):512 * (b + 2) + 512]

        if core == 0:
            xT = (x_sh.transpose(2, 1, 0)          # [F, T, B]
                  .reshape(2, 128, T, B)
                  .transpose(1, 2, 0, 3)).astype(BF16)  # [128, T, 2, 64]
        else:
            xT = zeros_xT

        kill = np.ones((128, NTICKS), np.float32)
        kill[:, :min(SKEW * core + 1, NTICKS)] = 0.0

        hmask = np.full((128, 1), 0.0 if core == 0 else 1.0, np.float32)
        ones01 = np.zeros((2, 128), np.float32)
        ones01[0, 0:64] = 1.0
        ones01[1, 64:128] = 1.0

        in_maps.append({
            "w_in_T": w_in_T, "w_hh_T": w_hh_T, "bias": bias.astype(BF16),
            "xT": xT, "kill": kill, "hmask": hmask,
            "ones01": ones01.astype(BF16),
        })
    return in_maps


def run_lstm(x_sh, weights, T=T_FULL, C=C_DEFAULT):
    """x_sh: [B, T, F] float32 (already teacher-forcing shifted).
    weights: list of L tuples (W_ih, W_hh, b_ih, b_hh)."""
    import os
    from concourse import bass2jax
    key = (T, C)
    if key not in _BUILD_CACHE:
        _BUILD_CACHE[key] = build(T, C)
    nc = _BUILD_CACHE[key]
    in_maps = _prep_core_inputs(x_sh, weights, T, C)
    if os.environ.get("BASS_LSTM_TRACE", "0") == "1":
        from concourse import bass_utils
        res = bass_utils.run_bass_kernel_spmd(
            nc, in_maps, core_ids=list(range(NCORES)), trace=True,
            tmpdir="/tmp/lstm_trace",
            trace_cores=[int(os.environ.get("BASS_LSTM_TRACE_CORE", "3"))])
        print("exec_time_ns:", res.exec_time_ns)
        print("profile_json:", res.profile_json)
        return res.results[L - 1]["out"]
    results = bass2jax.run_bass_via_pjrt(nc, in_maps, n_cores=NCORES)
    return results[L - 1]["out"]


def kernel(x, W_ih0, W_hh0, b_ih0, b_hh0, W_ih_rest, W_hh_rest, b_ih_rest,
           b_hh_rest, train_mode):
    x = np.asarray(x, dtype=np.float32)
    if int(train_mode):
        x_sh = np.concatenate(
            [np.zeros_like(x[:, :1]), x[:, :-1]], axis=1)
    else:
        x_sh = x
    weights = [(np.asarray(W_ih0, np.float32), np.asarray(W_hh0, np.float32),
                np.asarray(b_ih0, np.float32), np.asarray(b_hh0, np.float32))]
    for i in range(L - 1):
        weights.append((np.asarray(W_ih_rest[i], np.float32),
                        np.asarray(W_hh_rest[i], np.float32),
                        np.asarray(b_ih_rest[i], np.float32),
                        np.asarray(b_hh_rest[i], np.float32)))
    out = run_lstm(x_sh, weights, T=x.shape[1], C=C_DEFAULT)
    return np.asarray(out, dtype=np.float32)
